# revision 1
# baseline (speedup 1.0000x reference)
"""Trainium2 Bass kernel for a 2-layer GAT-style reduction network.

Reference math (per head h, per group of 16 nodes):
    wx   = x @ W[h]                                  # [*, n, d]
    z    = gelu(wx @ A_top[h] + wx[root] @ A_bot[h]) # root = node 0 of group
    att  = softmax(gelu(z), over n)
    out_h[g] = gelu(sum_n att[n] * wx[n])
    layer out = mean_h out_h
Applied twice: layer0 groups = N1(16) within (b, n2); layer1 groups = N2(16)
within b.

Key algebraic restructure used here:
  - sum_n att[n]*(x[n] @ W) == (sum_n att[n]*x[n]) @ W, so the big matmul
    runs per-group (1024 rows/core) instead of per-node (16384 rows/core).
  - wx @ A_top == x @ (W @ A_top) := x @ a_eff, so attention scores come from
    a thin [F, 8] matmul (a_eff/b_eff for 4 heads), not from wx.
  - head-mean of layer0 is absorbed into layer1 weights (0.25 * W1/aeff1);
    final 0.25 applied explicitly.

Distribution: pure data-parallel over 8 NeuronCores, sharding the batch
(512 -> 64 per core). Weights replicated. No collectives; host concatenates
the 8 output shards.

Dataflow per core (layer 0):
  A: DMA x blocks (512 tok) HBM->SBUF (f32r); DVE-cast to bf16; DMA bf16 to a
     DRAM bounce buffer (f-slice-major).
  B: HWDGE xbar-transposed reads give xT (bf16, [f, t]); thin zall matmul
     (a_eff) -> z scores [8, 512] per block; drain + pack into zbig [64, t]
     via partition-offset DMAs.
  C: z epilogue on packed [64, 512] tiles: z = za + zb_root(broadcast AP),
     gelu, gelu, exp (ACT, table-batched), segmented sum (16), reciprocal,
     att = e * recip (broadcast AP).
  D: att rows repacked to base-0, PE-transposed to [t, h]; S_att[t, (h,g)] =
     att * group-mask (one DVE op per block); stage-1 matmul with x tile as
     the stationary operand -> y^T accumulates in PSUM, drained to ybuf.
  E: stage-2 matmul W^T @ y^T (N=512) -> gelu -> head-sum -> x1^T (f32r).
Layer 1 repeats the same structure on x1^T (1024 tokens, already transposed
for the score path; PE-transposed back for stage-1).
"""

import sys

sys.path.insert(0, "/opt/trn_rl_repo")

import numpy as np
from contextlib import ExitStack

import concourse.bass as bass
import concourse.tile as tile
from concourse import bacc, mybir
from concourse.bass_utils import run_bass_kernel_spmd

dt = mybir.dt
AF = mybir.ActivationFunctionType

NCORES = 8
B, N2, N1, F, D, H = 512, 16, 16, 256, 256, 4
BS = B // NCORES  # 64 samples per core
T0 = BS * N2 * N1  # 16384 tokens, layer 0
NB0 = T0 // 1024  # 16 blocks of 1024 tokens
NCH = 4  # super-chunks (4 blocks each)
CB = NB0 // NCH  # 8 blocks per chunk
T1 = BS * N2  # 1024 tokens, layer 1
G0 = T0 // 16  # 1024 groups layer 0
G1 = T1 // 16  # 64 groups layer 1

TRACE = False  # set True (e.g. from test.py) to capture an NTFF profile
DEBUG = False  # add intermediate dram outputs for stage-wise HW debugging
_CACHE = {}


def f32(ap):
    return ap.bitcast(dt.float32)


def build_program():
    nc = bacc.Bacc("TRN2", target_bir_lowering=False, debug=False)

    x_d = nc.dram_tensor("x", [T0, F], dt.float32r, kind="ExternalInput").ap()
    w0_d = nc.dram_tensor("w0", [H, F, D], dt.float32r, kind="ExternalInput").ap()
    w1_d = nc.dram_tensor("w1", [H, D, D], dt.float32r, kind="ExternalInput").ap()
    aeb_d = nc.dram_tensor("aeb", [2, 128, 8], dt.bfloat16, kind="ExternalInput").ap()
    ae1_d = nc.dram_tensor("ae1", [2, 128, 8], dt.float32r, kind="ExternalInput").ap()
    mm_d = nc.dram_tensor("mmask", [128, 32], dt.float32, kind="ExternalInput").ap()
    id4_d = nc.dram_tensor("id4", [4, 4], dt.float32r, kind="ExternalInput").ap()
    id128_d = nc.dram_tensor("id128", [128, 128], dt.float32r, kind="ExternalInput").ap()
    out_d = nc.dram_tensor("out", [BS, D], dt.float32, kind="ExternalOutput").ap()
    if DEBUG:
        dbg_zbig = nc.dram_tensor("dbg_zbig", [64, 1024], dt.float32, kind="ExternalOutput").ap()
        dbg_att = nc.dram_tensor("dbg_att", [64, 512], dt.float32, kind="ExternalOutput").ap()
        dbg_ybuf = nc.dram_tensor("dbg_ybuf", [128, 2, 2048], dt.float32, kind="ExternalOutput").ap()
        dbg_x1t = nc.dram_tensor("dbg_x1t", [128, 2, 1024], dt.float32, kind="ExternalOutput").ap()
        dbg_xtb = nc.dram_tensor("dbg_xtb", [128, 2, 512], dt.bfloat16, kind="ExternalOutput").ap()
        dbg_zt0 = nc.dram_tensor("dbg_zt0", [8, 512], dt.float32, kind="ExternalOutput").ap()

    with tile.TileContext(nc) as tc, ExitStack() as ctx:
        cpool = ctx.enter_context(tc.tile_pool(name="consts", bufs=1))
        xpool = ctx.enter_context(tc.tile_pool(name="x", bufs=2))
        xbpool = ctx.enter_context(tc.tile_pool(name="xb", bufs=11))
        xtpool = ctx.enter_context(tc.tile_pool(name="xt", bufs=3))
        ztpool = ctx.enter_context(tc.tile_pool(name="zt", bufs=3))
        zbpool = ctx.enter_context(tc.tile_pool(name="zbig", bufs=2))
        epool = ctx.enter_context(tc.tile_pool(name="eps", bufs=2))
        atpool = ctx.enter_context(tc.tile_pool(name="att", bufs=2))
        abpool = ctx.enter_context(tc.tile_pool(name="attb", bufs=2))
        sapool = ctx.enter_context(tc.tile_pool(name="sab", bufs=3))
        ybpool = ctx.enter_context(tc.tile_pool(name="ybuf", bufs=1))
        ghpool = ctx.enter_context(tc.tile_pool(name="gh", bufs=3))
        adpool = ctx.enter_context(tc.tile_pool(name="ad", bufs=2))
        x1pool = ctx.enter_context(tc.tile_pool(name="x1", bufs=1))
        mpool = ctx.enter_context(tc.tile_pool(name="misc", bufs=1))

        ps_z = ctx.enter_context(tc.tile_pool(name="ps_z", bufs=2, space="PSUM"))
        ps_at = ctx.enter_context(tc.tile_pool(name="ps_at", bufs=2, space="PSUM"))
        ps_s1 = ctx.enter_context(tc.tile_pool(name="ps_s1", bufs=2, space="PSUM"))
        ps_s2 = ctx.enter_context(tc.tile_pool(name="ps_s2", bufs=2, space="PSUM"))

        # ---- constants ----
        w0_t = cpool.tile([128, H, 2, D], dt.float32r, tag="w0")
        nc.scalar.dma_start(out=w0_t[:], in_=w0_d.rearrange("h (fs p) d -> p h fs d", p=128))
        w1_t = cpool.tile([128, H, 2, D], dt.float32r, tag="w1")
        nc.scalar.dma_start(out=w1_t[:], in_=w1_d.rearrange("h (fs p) d -> p h fs d", p=128))
        aeb_t = cpool.tile([128, 2, 8], dt.bfloat16, tag="aeb")
        nc.scalar.dma_start(out=aeb_t[:], in_=aeb_d.rearrange("s p j -> p s j"))
        ae1_t = cpool.tile([128, 2, 8], dt.float32r, tag="ae1")
        nc.scalar.dma_start(out=ae1_t[:], in_=ae1_d.rearrange("s p j -> p s j"))
        mm_t = cpool.tile([128, 32], dt.float32, tag="mm")
        nc.scalar.dma_start(out=mm_t[:], in_=mm_d)
        id4_t = cpool.tile([4, 4], dt.float32r, tag="id4")
        nc.scalar.dma_start(out=id4_t[:], in_=id4_d)
        id128_t = cpool.tile([128, 128], dt.float32r, tag="id128")
        nc.scalar.dma_start(out=id128_t[:], in_=id128_d)


        # ================= LAYER 0 =================
        # Per-chunk pipeline. DMA queue assignment (queues are FIFO per
        # engine): sync = x loads + transposed reads + output; scalar =
        # bounce writes; gpsimd = small packing DMAs.
        x_tiles = {}
        x1T = [
            x1pool.tile([128, 1024], dt.float32r, tag=f"x1T{ds}", name=f"x1T{ds}")
            for ds in range(2)
        ]
        x1n_t = x1pool.tile([128, 8, 256], dt.float32r, tag="x1n", name="x1n")
        zt1 = [
            ztpool.tile([8, 512], dt.float32, tag="zt1", name=f"zt1_{lb}", bufs=2)
            for lb in range(2)
        ]
        for c in range(NCH):
            # ---- phase A+B: load, cast, transpose, zall, drain ----
            ztmps = []
            xbs = {}
            for bi in range(CB):
                b = c * CB + bi
                xt_ = xpool.tile([128, 8, F], dt.float32r, tag="x", name="xt_")
                nc.scalar.dma_start(
                    out=xt_[:],
                    in_=x_d[1024 * b : 1024 * (b + 1), :].rearrange(
                        "(n p) f -> p n f", p=128
                    ),
                )
                xb = xbpool.tile([128, 8, F], dt.bfloat16, tag="xb", name="xb")
                nc.vector.tensor_copy(xb[:], f32(xt_[:]))
                xbs[b] = xb
                xtb = xtpool.tile([128, 2, 1024], dt.bfloat16, tag="xt", name="xtb")
                for k in range(8):
                    for fs in range(2):
                        nc.sync.dma_start(
                            out=xtb[:, fs, 128 * k : 128 * (k + 1)],
                            in_=xb[:, k, 128 * fs : 128 * (fs + 1)],
                            transpose=True,
                        )
                zps = [None, None]
                for half in range(2):
                    zps[half] = ps_z.tile([8, 512], dt.float32, tag="zps", name="zps")
                    for fs in range(2):
                        nc.tensor.matmul(
                            zps[half][:],
                            aeb_t[:, fs, :],
                            xtb[:, fs, 512 * half : 512 * (half + 1)],
                            start=(fs == 0),
                            stop=(fs == 1),
                        )
                zt = ztpool.tile([8, 2, 512], dt.float32, tag="zt", name="zt")
                for half in range(2):
                    nc.scalar.copy(zt[:, half, :], zps[half][:])
                ztmps.append(zt)
                if DEBUG and b == 0:
                    nc.scalar.dma_start(out=dbg_zt0, in_=zt[:, 0, :])
                    nc.scalar.dma_start(out=dbg_xtb, in_=xtb[:, :, 0:512])

            # ---- phase B2: pack into zbig [32, 2048] (row 4*bp+h) ----
            zbig = zbpool.tile([16, 2048], dt.float32, tag="zbig", name="zbig")
            for bp in range(CB):
                zt = ztmps[bp]
                nc.gpsimd.dma_start(
                    out=zbig[4 * bp : 4 * bp + 4, 0:1024],
                    in_=zt[0:4, :, :],
                )
                nc.gpsimd.dma_start(
                    out=zbig[4 * bp : 4 * bp + 4, 1024:2048],
                    in_=zt[4:8, :, :],
                )
            if DEBUG and c == 0:
                nc.scalar.dma_start(out=dbg_zbig, in_=zbig[:])

            # ---- phase C: z epilogue on [32, 1024] ----
            zs = epool.tile([16, 1024], dt.float32, tag="epsA", name="zs")
            nc.vector.tensor_add(
                zs[:].rearrange("p (g j) -> p g j", j=16),
                zbig[:, 0:1024].rearrange("p (g j) -> p g j", j=16),
                zbig[:, 1024:2048]
                .rearrange("p (g j) -> p g j", j=16)[:, :, 0:1]
                .broadcast_to([16, 64, 16]),
            )
            g1 = epool.tile([16, 1024], dt.float32, tag="epsB", name="g1")
            nc.scalar.activation(g1[:], zs[:], AF.Gelu)
            sv = epool.tile([16, 1024], dt.float32, tag="epsA", name="sv")
            nc.scalar.activation(sv[:], g1[:], AF.Gelu)
            e = epool.tile([16, 1024], dt.float32, tag="epsB", name="e")
            nc.scalar.activation(e[:], sv[:], AF.Exp)
            den = mpool.tile([16, 64], dt.float32, tag=f"den{c}", name="den")
            nc.vector.reduce_sum(
                den[:].unsqueeze(2),
                e[:].rearrange("p (g j) -> p g j", j=16),
                axis=mybir.AxisListType.X,
            )
            rec = mpool.tile([16, 64], dt.float32, tag=f"rec{c}", name="rec")
            nc.vector.reciprocal(rec[:], den[:])
            att = atpool.tile([16, 1024], dt.float32r, tag="att", name="att")
            nc.vector.tensor_mul(
                att[:].rearrange("p (g j) -> p g j", j=16),
                e[:].rearrange("p (g j) -> p g j", j=16),
                rec[:].unsqueeze(2).broadcast_to([16, 64, 16]),
            )
            if DEBUG and c == 0:
                nc.scalar.dma_start(out=dbg_att, in_=f32(att[:]))

            # ---- phase D: att transpose + S_att + stage-1 (bf16) ----
            ybuf = ybpool.tile([128, 2, 1024], dt.float32r, tag="ybuf", name="ybuf")
            ybps = [None, None]
            for bp in range(CB):
                b = c * CB + bp
                attb = abpool.tile([4, 1024], dt.float32r, tag="attb", name="attb")
                nc.scalar.dma_start(out=attb[:], in_=att[4 * bp : 4 * bp + 4, :])
                atp = ps_at.tile([128, 32], dt.float32r, tag="atp", name="atp")
                for k in range(8):
                    nc.tensor.transpose(
                        atp[:, 4 * k : 4 * k + 4],
                        attb[0:4, 128 * k : 128 * (k + 1)],
                        id4_t[:],
                    )
                sab = sapool.tile([128, 8, 32], dt.bfloat16, tag="sab", name="sab")
                nc.vector.tensor_mul(
                    sab[:].rearrange("p k (h g) -> p k h g", g=8),
                    f32(atp[:])
                    .rearrange("p (k h) -> p k h", h=4)
                    .unsqueeze(3)
                    .broadcast_to([128, 8, 4, 8]),
                    mm_t[:]
                    .rearrange("p (h g) -> p h g", g=8)
                    .unsqueeze(1)
                    .broadcast_to([128, 8, 4, 8]),
                )
                for k in range(8):
                    K = b * 8 + k  # global x-tile index
                    kq = K % 16  # position within psum group
                    if kq == 0:
                        ybps = [
                            ps_s1.tile([128, 512], dt.float32, tag="ybps", name="ybps")
                            for _ in range(2)
                        ]
                    for fs in range(2):
                        nc.tensor.matmul(
                            ybps[fs][:, 32 * kq : 32 * kq + 32],
                            xbs[b][:, k, 128 * fs : 128 * (fs + 1)],
                            sab[:, k, :],
                            start=(kq == 0),
                            stop=(kq == 15),
                        )
                    if kq == 15:
                        q = (K % 32) // 16
                        for fs in range(2):
                            nc.vector.tensor_copy(
                                ybuf[:, fs, 512 * q : 512 * (q + 1)],
                                ybps[fs][:],
                            )
            if DEBUG and c == 0:
                nc.scalar.dma_start(out=dbg_ybuf, in_=f32(ybuf[:]))

            # ---- phase E: stage-2 + gelu head-sum ----
            for ds in range(2):
                ghs = []
                for h in range(H):
                    o2 = ps_s2.tile([128, 256], dt.float32, tag="o2", name="o2")
                    for fs in range(2):
                        nc.tensor.matmul(
                            o2[:],
                            w0_t[:, h, fs, 128 * ds : 128 * (ds + 1)],
                            ybuf[:, fs, :].rearrange(
                                "p (K hh g) -> p K hh g", hh=4, g=8
                            )[:, :, h, :],
                            start=(fs == 0),
                            stop=(fs == 1),
                        )
                    gh = ghpool.tile([128, 256], dt.float32, tag="gh", name="gh")
                    nc.scalar.activation(gh[:], o2[:], AF.Gelu)
                    ghs.append(gh)
                ad1 = adpool.tile([128, 256], dt.float32, tag="ad", name="ad1")
                nc.vector.tensor_add(ad1[:], ghs[0][:], ghs[1][:])
                ad2 = adpool.tile([128, 256], dt.float32, tag="ad", name="ad2")
                nc.vector.tensor_add(ad2[:], ghs[2][:], ghs[3][:])
                nc.vector.tensor_add(
                    x1T[ds][:, 256 * c : 256 * (c + 1)], ad1[:], ad2[:]
                )

            # hoisted layer-1 prep: transpose this chunk's x1T cols; zall1
            # once a 512-col half completes
            for j in (2 * c, 2 * c + 1):
                for ds in range(2):
                    trp = ps_at.tile([128, 128], dt.float32r, tag="atp", name="trp")
                    nc.tensor.transpose(
                        trp[:], x1T[ds][:, 128 * j : 128 * (j + 1)], id128_t[:]
                    )
                    nc.vector.tensor_copy(
                        x1n_t[:, j, 128 * ds : 128 * (ds + 1)], f32(trp[:])
                    )
            if c % 2 == 1:
                lb = c // 2
                z1ps = ps_z.tile([8, 512], dt.float32, tag="zps", name="z1ps")
                for ds in range(2):
                    nc.tensor.matmul(
                        z1ps[:],
                        ae1_t[:, ds, :],
                        x1T[ds][:, 512 * lb : 512 * (lb + 1)],
                        start=(ds == 0),
                        stop=(ds == 1),
                    )
                nc.scalar.copy(zt1[lb][:], z1ps[:])

        # ================= LAYER 1 =================
        # tokens t2 = 1024 (8 tiles), groups of 16, roots at t2 % 16 == 0
        # (zall1 + x1n hoisted into layer-0 phase E)
        zb1 = zbpool.tile([64, 1024], dt.float32, tag="zbig", name="zb1")
        for h in range(H):
            for lb in range(2):
                nc.scalar.dma_start(
                    out=zb1[2 * h + lb : 2 * h + lb + 1, 0:512],
                    in_=zt1[lb][h : h + 1, :],
                )
                nc.scalar.dma_start(
                    out=zb1[2 * h + lb : 2 * h + lb + 1, 512:1024],
                    in_=zt1[lb][4 + h : 5 + h, :],
                )
        zs1 = epool.tile([8, 512], dt.float32, tag="epsA", name="zs1")
        nc.vector.tensor_add(
            zs1[:].rearrange("p (g j) -> p g j", j=16),
            zb1[0:8, 0:512].rearrange("p (g j) -> p g j", j=16),
            zb1[0:8, 512:1024]
            .rearrange("p (g j) -> p g j", j=16)[:, :, 0:1]
            .broadcast_to([8, 32, 16]),
        )
        g11 = epool.tile([8, 512], dt.float32, tag="epsB", name="g11")
        nc.scalar.activation(g11[:], zs1[:], AF.Gelu)
        s1t = epool.tile([8, 512], dt.float32, tag="epsA", name="s1t")
        nc.scalar.activation(s1t[:], g11[:], AF.Gelu)
        e1 = epool.tile([8, 512], dt.float32, tag="epsB", name="e1")
        nc.scalar.activation(e1[:], s1t[:], AF.Exp)
        den1 = mpool.tile([8, 32], dt.float32, tag="den1")
        nc.vector.reduce_sum(
            den1[:].unsqueeze(2),
            e1[:].rearrange("p (g j) -> p g j", j=16),
            axis=mybir.AxisListType.X,
        )
        rec1 = mpool.tile([8, 32], dt.float32, tag="rec1")
        nc.vector.reciprocal(rec1[:], den1[:])
        att1f = mpool.tile([8, 512], dt.float32r, tag="att1", name="att1")
        att1 = att1f[0:8, :]
        nc.vector.tensor_mul(
            att1.rearrange("p (g j) -> p g j", j=16),
            e1[:].rearrange("p (g j) -> p g j", j=16),
            rec1[:].unsqueeze(2).broadcast_to([8, 32, 16]),
        )
        attb1 = abpool.tile([4, 1024], dt.float32r, tag="attb1", bufs=1)
        for h in range(H):
            nc.scalar.dma_start(
                out=attb1[h : h + 1, :].rearrange("p (lb t) -> p lb t", t=512),
                in_=att1f[2 * h : 2 * h + 2, :],
            )

        # ---- stage-1 layer 1 ----
        y1ps = [
            ps_s1.tile([128, 256], dt.float32, tag="ybps", name="y1ps")
            for _ in range(2)
        ]
        for j in range(8):
            atp1 = ps_at.tile([128, 4], dt.float32r, tag="atp")
            nc.tensor.transpose(
                atp1[:],
                attb1[0:4, 512 * (j // 4) + 128 * (j % 4) : 512 * (j // 4) + 128 * (j % 4) + 128],
                id4_t[:],
            )
            sab1 = sapool.tile([128, 32], dt.float32r, tag="sab1")
            nc.vector.tensor_mul(
                sab1[:].rearrange("p (h g) -> p h g", g=8),
                f32(atp1[:]).unsqueeze(2).broadcast_to([128, 4, 8]),
                mm_t[:].rearrange("p (h g) -> p h g", g=8),
            )
            for ds in range(2):
                nc.tensor.matmul(
                    y1ps[ds][:, 32 * j : 32 * j + 32],
                    x1n_t[:, j, 128 * ds : 128 * (ds + 1)],
                    sab1[:],
                    start=(j == 0),
                    stop=(j == 7),
                )
        y1b = []
        for ds in range(2):
            yb = mpool.tile([128, 256], dt.float32r, tag=f"y1b{ds}")
            nc.vector.tensor_copy(yb[:], y1ps[ds][:])
            y1b.append(yb)

        # ---- stage-2 layer 1 + final ----
        out_sb = mpool.tile([64, 256], dt.float32, tag="out_sb")
        for d2s in range(2):
            ghs = []
            for h in range(H):
                o21 = ps_s2.tile([128, 64], dt.float32, tag="o2")
                for ds in range(2):
                    nc.tensor.matmul(
                        o21[:],
                        w1_t[:, h, ds, 128 * d2s : 128 * (d2s + 1)],
                        y1b[ds][:].rearrange("p (j hh g) -> p j hh g", hh=4, g=8)[
                            :, :, h, :
                        ],
                        start=(ds == 0),
                        stop=(ds == 1),
                    )
                gh = ghpool.tile([128, 64], dt.float32, tag="gh1")
                nc.scalar.activation(gh[:], o21[:], AF.Gelu)
                ghs.append(gh)
            ad1 = adpool.tile([128, 64], dt.float32, tag="ad1")
            nc.vector.tensor_add(ad1[:], ghs[0][:], ghs[1][:])
            ad2 = adpool.tile([128, 64], dt.float32, tag="ad1")
            nc.vector.tensor_add(ad2[:], ghs[2][:], ghs[3][:])
            u = mpool.tile([128, 64], dt.float32, tag=f"u{d2s}")
            nc.vector.tensor_add(u[:], ad1[:], ad2[:])
            uT = mpool.tile([128, 64], dt.float32r, tag=f"uT{d2s}")
            nc.vector.tensor_scalar_mul(uT[:], u[:], 0.25)
            otp = ps_at.tile([64, 128], dt.float32r, tag="atp")
            nc.tensor.transpose(otp[:], uT[:], id128_t[:])
            nc.vector.tensor_copy(out_sb[:, 128 * d2s : 128 * (d2s + 1)], f32(otp[:]))
        nc.scalar.dma_start(out=out_d, in_=out_sb[:])

    nc.compile()
    return nc


def _prep_weights(W0, A0, W1, A1):
    import ml_dtypes

    def effs(W, A):
        # a_eff[h] = W[h] @ A[h,:256,0]; b_eff[h] = W[h] @ A[h,256:,0]
        a = np.einsum("hfd,hd->hf", W.astype(np.float64), A[:, :256, 0].astype(np.float64))
        b = np.einsum("hfd,hd->hf", W.astype(np.float64), A[:, 256:, 0].astype(np.float64))
        # cols j: 0..3 = a_eff per head, 4..7 = b_eff per head -> [F, 8]
        return np.concatenate([a.T, b.T], axis=1).astype(np.float32)

    ae0 = effs(W0, A0)  # [256, 8]
    ae1 = 0.25 * effs(W1, A1)  # [256, 8]
    aeb = ae0.reshape(2, 128, 8).astype(ml_dtypes.bfloat16)
    ae1r = np.ascontiguousarray(ae1.reshape(2, 128, 8))
    w1s = (0.25 * W1).astype(np.float32)

    t = np.arange(128)
    c = np.arange(32)
    mmask = ((c[None, :] % 8) == (t[:, None] // 16)).astype(np.float32)
    id4 = np.eye(4, dtype=np.float32)
    id128 = np.eye(128, dtype=np.float32)
    return {
        "w0": np.ascontiguousarray(W0.astype(np.float32)),
        "w1": np.ascontiguousarray(w1s),
        "aeb": np.ascontiguousarray(aeb),
        "ae1": ae1r.astype(np.float32),
        "mmask": mmask,
        "id4": id4,
        "id128": id128,
    }


def kernel(x, W0, A0, W1, A1):
    x = np.asarray(x, dtype=np.float32)
    W0 = np.asarray(W0, dtype=np.float32)
    A0 = np.asarray(A0, dtype=np.float32)
    W1 = np.asarray(W1, dtype=np.float32)
    A1 = np.asarray(A1, dtype=np.float32)

    if "nc" not in _CACHE:
        _CACHE["nc"] = build_program()
    nc = _CACHE["nc"]

    wmap = _prep_weights(W0, A0, W1, A1)
    xs = x.reshape(NCORES, T0, F)
    in_maps = [dict(wmap, x=np.ascontiguousarray(xs[i])) for i in range(NCORES)]
    res = run_bass_kernel_spmd(
        nc, in_maps, core_ids=list(range(NCORES)), trace=TRACE
    )
    _CACHE["last_result"] = res
    out = np.concatenate([res.results[i]["out"] for i in range(NCORES)], axis=0)
    return out

